# revision 3
# baseline (speedup 1.0000x reference)
# Trainium2 Bass kernel for nn_MultiHeadAttention_48533130445634.
#
# Math (faithful to the reference, including its unusual second einsum):
#   scores[b,h,n,m] = softmax_m( (q[b,h,n,:] . k[b,h,m,:]) * 0.125 )
#   out[b,h,m,d]    = (sum_n scores[b,h,n,m]) * v[b,h,m,d]
#
# i.e. the output is V scaled elementwise by the column-sums of the softmax
# matrix.  Per (b,h), tiled over n (128 rows at a time):
#   S_i = Q_i K^T            (PE, fp32r, PSUM out, 1024-wide halves)
#   E_i = exp(S_i * 0.125)   split across two engines:
#     ACT tiles: scalar-engine exp (bf16 out to SBUF) with the ACT
#       accumulator emitting the per-half row-sum for free.
#     DVE tiles: Schraudolph-style exp on the vector engine -- one
#       tensor_scalar (S*A + B) written through an int16 bitcast of the bf16
#       E tile; the integer lands in the bf16 exponent/mantissa fields so the
#       bits ARE ~exp(S*0.125).  GpSimd adds the two m-halves, DVE reduces
#       the sum to the row-sum.  This offloads ~40% of the exp roofline off
#       the scalar engine.
#   g_i = 1 / rowsum_i       (DVE reciprocal, batched by 4 tiles)
#   colsum += g_i^T @ E_i    (PE, bf16; accumulated in one PSUM bank using
#                             four output base-partitions 0/32/64/96, one
#                             per 512-wide m-chunk)
#   out[m,d] = colsum[m] * v[m,d]   (GpSimd tensor_scalar)
#
# Colsum matmuls are queued and drip-fed (one tile's worth per n-tile slot,
# only once their g is ready) so they never head-of-line-block the S fills.
# The m index maps to partitions as m = 16p + t so V loads, output stores
# and the colsum scatter are all contiguous per partition.
#
# Sharding: 64 (b,h) pairs split across 8 cores, 8 pairs each (SPMD, no
# cross-core communication).  Q/K are pre-transposed on the host so the
# contraction dim (Dh=64) lands on SBUF partitions for the PE.

import math
import os

import numpy as np

import concourse.mybir as mybir
import concourse.tile as tile
from concourse import bacc
from concourse.bass_utils import run_bass_kernel_spmd

B, H, N, D = 4, 16, 2048, 64
N_CORES = 8
H_LOC = (B * H) // N_CORES  # 8 (b,h) pairs per core
P = 128                     # partition tile along n
NT = N // P                 # 16 n-tiles
SCALE = 0.125               # (DIM // N_HEADS) ** -0.5
MH = 2                      # m processed in halves of 1024 (PSUM bank budget)
MW = N // MH                # 1024

# Schraudolph exp for bf16: bitcast(int16(A*S + B)) ~= exp(S*SCALE).
SCH_A = SCALE * 128.0 / math.log(2.0)
SCH_B = 127.0 * 128.0 - 4.0

# n-tiles (within a head) whose exp runs on the vector engine via the
# Schraudolph trick; the rest run on the scalar engine with the ACT
# accumulator producing the row-sum.  Avoid batch-final tiles (3/7/11/15)
# since the DVE path's row-sum lands ~2 slots later, and avoid tiles 0/15
# so GpSimd pair-adds don't collide with the per-head output multiply.
DVE_TILES = frozenset({1, 2, 5, 6, 9, 10})

f32 = mybir.dt.float32
f32r = mybir.dt.float32r
bf16 = mybir.dt.bfloat16
i16 = mybir.dt.int16
Exp = mybir.ActivationFunctionType.Exp
Add = mybir.AluOpType.add
Mult = mybir.AluOpType.mult


def _attention_kernel(tc, out, qT, kT, vin):
    nc = tc.nc

    with (
        tc.tile_pool(name="qk", bufs=3) as qk_pool,
        tc.tile_pool(name="ev", bufs=14) as e_pool,
        tc.tile_pool(name="vo", bufs=4) as vo_pool,
        tc.tile_pool(name="st", bufs=2) as st_pool,
        tc.tile_pool(name="tm", bufs=3) as tmp_pool,
        tc.tile_pool(name="s_ps", bufs=3, space="PSUM") as s_pool,
        tc.tile_pool(name="c_ps", bufs=2, space="PSUM") as c_pool,
    ):
        # Preload the exp table set and start the PE p-state ramp while the
        # first DMAs are in flight.
        warm = st_pool.tile([P, 1], f32, tag="warm")
        nc.gpsimd.memset(warm[:, :], 0.0)
        nc.scalar.activation(warm[:, :], warm[:, :], func=Exp)
        warm_ps = c_pool.tile([1, 1], f32, tag="csum")
        nc.tensor.matmul(
            warm_ps[0:1, 0:1], lhsT=warm[0:1, 0:1], rhs=warm[0:1, 0:1],
            start=True, stop=True, skip_group_check=True,
        )

        # Q/K/V loads for head h, emitted one head ahead so the SP sequencer
        # issues them before it blocks on the previous head's tail DMAs.
        loaded = {}

        def emit_loads(h, first=False):
            q_s = qk_pool.tile([D, N], f32r, tag="q")
            k_s = qk_pool.tile([D, N], f32r, tag="k")
            if first:
                # order so the very first S fill's operands land earliest
                parts = [(k_s, kT, 0, 512), (q_s, qT, 0, 512),
                         (k_s, kT, 512, MW), (k_s, kT, MW, N),
                         (q_s, qT, 512, MW), (q_s, qT, MW, N)]
                for t_s, src, lo, hi in parts:
                    nc.sync.dma_start(out=t_s[:, lo:hi], in_=src[h, :, lo:hi])
            else:
                for half in range(2):
                    sl = slice(half * MW, (half + 1) * MW)
                    nc.sync.dma_start(out=k_s[:, sl], in_=kT[h, :, sl])
                    nc.sync.dma_start(out=q_s[:, sl], in_=qT[h, :, sl])
            v_s = vo_pool.tile([P, NT, D], f32, tag="v")
            nc.sync.dma_start(
                out=v_s[:, :, :], in_=vin[h].rearrange("(p t) d -> p t d", p=P)
            )
            # prepare the colsum accumulator a head ahead too, so its
            # memset (DVE) and PSUM WAR never stall the head boundary
            c_ps = c_pool.tile([P, 512], f32, tag="csum")
            nc.vector.memset(c_ps[:, :], 0.0)
            loaded[h] = (q_s, k_s, v_s, c_ps)

        emit_loads(0, first=True)

        # colsum matmuls pending emission: (min_slot, j, c_ps, g_bf, e_j, tail)
        pending = []
        # DVE-tile row-sum reduces pending emission: (min_slot, fn)
        red_pending = []
        slot = 0

        def emit_colsum(entry, c_lo, c_hi):
            _, j, c_ps, g_bf, e_j, tail_fn = entry
            for c in range(c_lo, c_hi):
                nc.tensor.matmul(
                    c_ps[32 * c : 32 * c + 1, :],
                    lhsT=g_bf[:, j : j + 1],
                    rhs=e_j[:, c * 512 : (c + 1) * 512],
                    start=(j == 0),
                    stop=(j == NT - 1),
                    skip_group_check=True,
                    tile_position=(0, 32 * c),
                )
            if c_hi == N // 512 and tail_fn is not None:
                tail_fn()

        for h in range(H_LOC):
            last_head = h == H_LOC - 1
            q_s, k_s, v_s, c_ps = loaded.pop(h)
            if not last_head:
                emit_loads(h + 1)

            # per-tile rowsum parts: [:, i, 0] and [:, i, 1] summed later
            rs_parts = st_pool.tile([P, NT, 2], f32, tag="rsp")
            nc.gpsimd.memset(rs_parts[:, :, :], 0.0)
            rowsum = st_pool.tile([P, NT], f32, tag="rowsum")
            g = st_pool.tile([P, NT], f32, tag="g")
            g_bf = st_pool.tile([P, NT], bf16, tag="gbf")
            e_tiles = []

            def make_tail(h=h, c_ps=c_ps, v_s=v_s, last_head=last_head):
                def tail():
                    # colsum [4 x 512 at partitions 0/32/64/96] -> csT [P, NT].
                    # m = 16p + t, so chunk-major cs4 [4x512] and csT [128,16]
                    # walk m in the same order: one direct SBUF->SBUF DMA.
                    cs4 = st_pool.tile([P, 512], f32, tag="cs4")
                    nc.vector.tensor_copy(cs4[:, :], c_ps[:, :])
                    csT = st_pool.tile([P, NT], f32, tag="csT")
                    nc.sync.dma_start(out=csT[:, :], in_=cs4[0:P:32, :])
                    o_s = vo_pool.tile([P, NT, D], f32, tag="o")
                    eng = nc.vector if last_head else nc.gpsimd
                    out_r = out[h].rearrange("(p t) d -> p t d", p=P)
                    halves = ((0, NT // 2), (NT // 2, NT)) if last_head else ((0, NT),)
                    for t0, t1 in halves:
                        eng.tensor_tensor(
                            o_s[:, t0:t1, :],
                            v_s[:, t0:t1, :],
                            csT[:, t0:t1].unsqueeze(-1).broadcast_to((P, t1 - t0, D)),
                            op=Mult,
                        )
                        nc.sync.dma_start(
                            out=out_r[:, t0:t1, :], in_=o_s[:, t0:t1, :]
                        )

                return tail

            tail_fn = make_tail()

            # g-batches: (first_tile, last_tile inclusive); the last head
            # finishes with micro-batches so colsum work drains in-loop
            if last_head:
                batches = [(0, 3), (4, 7), (8, 11), (12, 13), (14, 14), (15, 15)]
            else:
                batches = [(b0, b0 + 3) for b0 in range(0, NT, 4)]
            batch_of = {}
            for b0, b1 in batches:
                for j in range(b0, b1 + 1):
                    batch_of[j] = (b0, b1)

            # rowsum state per batch: slot at which the batch's parts are all
            # emitted (used to derive the colsum drip lag)
            dve_in_batch = {
                (b0, b1): [j for j in range(b0, b1 + 1) if j in DVE_TILES]
                for b0, b1 in batches
            }

            for i in range(NT):
                slot += 1
                use_dve = i in DVE_TILES
                e_i = e_pool.tile([P, N], bf16, tag="e")
                e_tiles.append(e_i)
                # drip-feed pending colsum matmuls: half a tile's worth
                # after each m-half, so PE bursts never delay the S fills
                entry = None
                for mh in range(MH):
                    s_ps = s_pool.tile([P, MW], f32, tag="s")
                    for c in range(MW // 512):
                        m0 = mh * MW + c * 512
                        nc.tensor.matmul(
                            s_ps[:, c * 512 : (c + 1) * 512],
                            lhsT=q_s[:, i * P : (i + 1) * P],
                            rhs=k_s[:, m0 : m0 + 512],
                            start=True,
                            stop=True,
                        )
                    if use_dve:
                        nc.vector.tensor_scalar(
                            e_i[:, mh * MW : (mh + 1) * MW].bitcast(i16),
                            s_ps[:, :],
                            SCH_A,
                            SCH_B,
                            op0=Mult,
                            op1=Add,
                        )
                    else:
                        nc.scalar.activation(
                            e_i[:, mh * MW : (mh + 1) * MW],
                            s_ps[:, :],
                            func=Exp,
                            scale=SCALE,
                            accum_out=rs_parts[:, i, mh : mh + 1],
                        )
                    if mh == 0:
                        if pending and pending[0][0] <= slot:
                            entry = pending.pop(0)
                        if entry is not None:
                            emit_colsum(entry, 0, 2)
                    elif entry is not None:
                        emit_colsum(entry, 2, 4)

                if use_dve:
                    # GpSimd folds the two halves; the DVE reduce is deferred
                    # a slot so it doesn't head-of-line-block the next exp.
                    tmp = tmp_pool.tile([P, MW], bf16, tag="t")
                    nc.gpsimd.tensor_tensor(
                        tmp[:, :], e_i[:, 0:MW], e_i[:, MW:N], op=Add
                    )

                    def emit_reduce(i=i, tmp=tmp, rs_parts=rs_parts):
                        nc.vector.tensor_reduce(
                            rs_parts[:, i, 0:1],
                            tmp[:, :],
                            axis=mybir.AxisListType.X,
                            op=Add,
                        )

                    red_pending.append((slot + 1, emit_reduce))

                # flush any due row-sum reduces before the batch boundary
                while red_pending and red_pending[0][0] <= slot:
                    red_pending.pop(0)[1]()

                if i == batch_of[i][1]:  # batch boundary: g for the batch
                    b0, b1 = batch_of[i]
                    # make sure every reduce this batch needs is emitted
                    while red_pending and red_pending[0][0] <= slot + 1:
                        red_pending.pop(0)[1]()
                    sl = slice(b0, b1 + 1)
                    nc.vector.tensor_tensor(
                        rowsum[:, sl],
                        rs_parts[:, sl, 0],
                        rs_parts[:, sl, 1],
                        op=Add,
                    )
                    nc.vector.reciprocal(g[:, sl], rowsum[:, sl])
                    nc.vector.tensor_copy(g_bf[:, sl], g[:, sl])
                    # colsum drip lag: +1 slot when the batch ends on the ACT
                    # accumulator path, +2 when a DVE reduce gates it.
                    lag = 2 if dve_in_batch[(b0, b1)] else 1
                    for idx, j in enumerate(range(b0, b1 + 1)):
                        pending.append(
                            (
                                slot + lag + idx,
                                j,
                                c_ps,
                                g_bf,
                                e_tiles[j],
                                tail_fn if j == NT - 1 else None,
                            )
                        )

            if last_head:
                while red_pending:
                    red_pending.pop(0)[1]()
                while pending:
                    emit_colsum(pending.pop(0), 0, N // 512)


_NC_CACHE = None


def _get_nc():
    global _NC_CACHE
    if _NC_CACHE is None:
        nc = bacc.Bacc("TRN2", target_bir_lowering=False, debug=False)
        qT = nc.dram_tensor("qT", [H_LOC, D, N], f32r, kind="ExternalInput").ap()
        kT = nc.dram_tensor("kT", [H_LOC, D, N], f32r, kind="ExternalInput").ap()
        vin = nc.dram_tensor("v", [H_LOC, N, D], f32, kind="ExternalInput").ap()
        out = nc.dram_tensor("out", [H_LOC, N, D], f32, kind="ExternalOutput").ap()
        with tile.TileContext(nc) as tc:
            _attention_kernel(tc, out, qT, kT, vin)
        nc.compile()
        _NC_CACHE = nc
    return _NC_CACHE


def kernel(q, k, v):
    q = np.asarray(q, dtype=np.float32).reshape(B * H, N, D)
    k = np.asarray(k, dtype=np.float32).reshape(B * H, N, D)
    v = np.asarray(v, dtype=np.float32).reshape(B * H, N, D)

    in_maps = []
    for c in range(N_CORES):
        sl = slice(H_LOC * c, H_LOC * (c + 1))
        in_maps.append(
            {
                "qT": np.ascontiguousarray(q[sl].transpose(0, 2, 1)),
                "kT": np.ascontiguousarray(k[sl].transpose(0, 2, 1)),
                "v": np.ascontiguousarray(v[sl]),
            }
        )

    trace = bool(os.environ.get("KERNEL_TRACE"))
    res = run_bass_kernel_spmd(
        _get_nc(), in_maps, core_ids=list(range(N_CORES)), trace=trace
    )
    if trace:
        print(f"HW exec time: {res.exec_time_ns} ns")
        if res.instructions_and_trace is not None:
            print(f"trace: {res.instructions_and_trace[1]}")

    outs = [r["out"] for r in res.results]
    return np.concatenate(outs, axis=0).reshape(B, H, N, D)


# revision 36
# speedup vs baseline: 1.1659x; 1.1659x over previous
# Trainium2 Bass kernel for nn_MultiHeadAttention_48533130445634.
#
# Math (faithful to the reference, including its unusual second einsum):
#   scores[b,h,n,m] = softmax_m( (q[b,h,n,:] . k[b,h,m,:]) * 0.125 )
#   out[b,h,m,d]    = (sum_n scores[b,h,n,m]) * v[b,h,m,d]
#
# i.e. the output is V scaled elementwise by the column-sums of the softmax
# matrix.  Per (b,h), tiled over n (128 rows at a time):
#   S_i = Q_i K^T            (PE, fp32r, PSUM out, 1024-wide halves)
#   E_i = exp(S_i * 0.125)   split across two engines:
#     ACT tiles: scalar-engine exp (bf16 out to SBUF) with the ACT
#       accumulator emitting the per-half row-sum for free.
#     DVE tiles: Schraudolph-style exp on the vector engine -- one
#       tensor_scalar (S*A + B) written through an int16 bitcast of the bf16
#       E tile; the integer lands in the bf16 exponent/mantissa fields so the
#       bits ARE ~exp(S*0.125).  GpSimd adds the two m-halves, DVE reduces
#       the sum to the row-sum.  This offloads ~40% of the exp roofline off
#       the scalar engine.
#   g_i = 1 / rowsum_i       (DVE reciprocal, batched by 4 tiles)
#   colsum += g_i^T @ E_i    (PE, bf16; accumulated in one PSUM bank using
#                             four output base-partitions 0/32/64/96, one
#                             per 512-wide m-chunk)
#   out[m,d] = colsum[m] * v[m,d]   (GpSimd tensor_scalar)
#
# Colsum matmuls are queued and drip-fed (one tile's worth per n-tile slot,
# only once their g is ready) so they never head-of-line-block the S fills.
# The m index maps to partitions as m = 16p + t so V loads, output stores
# and the colsum scatter are all contiguous per partition.
#
# Sharding: 64 (b,h) pairs split across 8 cores, 8 pairs each (SPMD, no
# cross-core communication).  Q/K are pre-transposed on the host so the
# contraction dim (Dh=64) lands on SBUF partitions for the PE.

import math
import os

import numpy as np

import concourse.mybir as mybir
import concourse.tile as tile
from concourse import bacc
from concourse.bass_utils import run_bass_kernel_spmd

B, H, N, D = 4, 16, 2048, 64
N_CORES = 8
H_LOC = (B * H) // N_CORES  # 8 (b,h) pairs per core
P = 128                     # partition tile along n
NT = N // P                 # 16 n-tiles
SCALE = 0.125               # (DIM // N_HEADS) ** -0.5
MH = 2                      # m processed in halves of 1024 (PSUM bank budget)
MW = N // MH                # 1024

# Schraudolph exp for bf16: bitcast(int16(A*S + B)) ~= exp(S*SCALE).
SCH_A = SCALE * 128.0 / math.log(2.0)
SCH_B = 127.0 * 128.0 - 4.0

# n-tiles (within a head) whose exp runs on the vector engine via the
# Schraudolph trick; the rest run on the scalar engine with the ACT
# accumulator producing the row-sum.  Evenly interleaved, at g-pair starts
# (so the slower DVE row-sum path hides in slots where the DVE has no exp
# work), keeping ACT runs short; the last head keeps its final tiles on the
# fast ACT path so the tail drains quickly.
DVE_TILES = frozenset({2, 4, 6, 8, 12, 14})
DVE_TILES_LAST = frozenset({2, 4, 6, 8, 10, 12})

f32 = mybir.dt.float32
f32r = mybir.dt.float32r
bf16 = mybir.dt.bfloat16
i16 = mybir.dt.int16
f8e4 = mybir.dt.float8e4
Exp = mybir.ActivationFunctionType.Exp
Add = mybir.AluOpType.add
Mult = mybir.AluOpType.mult
DoubleRow = mybir.MatmulPerfMode.DoubleRow


def _attention_kernel(tc, out, qT, kT, vin):
    nc = tc.nc

    with (
        tc.tile_pool(name="qk", bufs=3) as qk_pool,
        tc.tile_pool(name="ev", bufs=14) as e_pool,
        tc.tile_pool(name="vo", bufs=4) as vo_pool,
        tc.tile_pool(name="st", bufs=2) as st_pool,
        tc.tile_pool(name="tm", bufs=3) as tmp_pool,
        tc.tile_pool(name="s_ps", bufs=3, space="PSUM") as s_pool,
        tc.tile_pool(name="c_ps", bufs=2, space="PSUM") as c_pool,
    ):
        # Preload the exp table set and start the PE p-state ramp while the
        # first DMAs are in flight.
        warm = st_pool.tile([P, 1], f32, tag="warm")
        nc.gpsimd.memset(warm[:, :], 0.0)
        nc.scalar.activation(warm[:, :], warm[:, :], func=Exp)
        warm_ps = c_pool.tile([1, 1], f32, tag="csum")
        nc.tensor.matmul(
            warm_ps[0:1, 0:1], lhsT=warm[0:1, 0:1], rhs=warm[0:1, 0:1],
            start=True, stop=True, skip_group_check=True,
        )

        # Q/K/V loads for head h, emitted one head ahead so the SP sequencer
        # issues them before it blocks on the previous head's tail DMAs.
        loaded = {}

        def emit_loads(h, first=False):
            # q/k in fp8e4m3, laid out [32, 2, N]: contraction d = 2*dp + j
            # so the S matmul runs in DoubleRow mode (2 fp8 weights per PE
            # cell -> half the cycles per output row)
            q_s = qk_pool.tile([D // 2, 2, N], f8e4, tag="q")
            k_s = qk_pool.tile([D // 2, 2, N], f8e4, tag="k")
            if first:
                # order so the very first S fill's operands land earliest
                parts = [(k_s, kT, 0, 512), (q_s, qT, 0, 512),
                         (k_s, kT, 512, MW), (k_s, kT, MW, N),
                         (q_s, qT, 512, MW), (q_s, qT, MW, N)]
                for t_s, src, lo, hi in parts:
                    nc.sync.dma_start(
                        out=t_s[:, :, lo:hi], in_=src[h, :, :, lo:hi]
                    )
            else:
                for half in range(2):
                    sl = slice(half * MW, (half + 1) * MW)
                    nc.sync.dma_start(out=k_s[:, :, sl], in_=kT[h, :, :, sl])
                    nc.sync.dma_start(out=q_s[:, :, sl], in_=qT[h, :, :, sl])
            v_s = vo_pool.tile([P, NT, D], f32, tag="v")
            nc.sync.dma_start(
                out=v_s[:, :, :], in_=vin[h].rearrange("(p t) d -> p t d", p=P)
            )
            # the colsum accumulator needs no memset: the first matmul into
            # each bank-chunk has start=True (overwrite), and only the four
            # written rows are ever read back
            c_ps = c_pool.tile([P, 512], f32, tag="csum")
            loaded[h] = (q_s, k_s, v_s, c_ps)

        emit_loads(0, first=True)

        # colsum matmuls pending emission: [min_slot, chunks_done, j, c_ps,
        # g_bf, e_j, tail]
        pending = []
        # DVE-tile row-sum reduces pending emission: (min_slot, fn)
        red_pending = []
        slot = 0

        def drip_colsum(budget):
            """Emit up to `budget` pending colsum chunk matmuls whose due
            slot has arrived.  Chunk-granular so PE colsum work interleaves
            tightly between S fills instead of bursting ahead of them."""
            while budget > 0 and pending and pending[0][0] <= slot:
                entry = pending[0]
                _, done, j, c_ps, g_bf, e_j, tail_fn = entry
                take = min(budget, 4 - done)
                for c in range(done, done + take):
                    nc.tensor.matmul(
                        c_ps[32 * c : 32 * c + 1, :],
                        lhsT=g_bf[:, j : j + 1],
                        rhs=e_j[:, c * 512 : (c + 1) * 512],
                        start=(j == 0),
                        stop=(j == NT - 1),
                        skip_group_check=True,
                        tile_position=(0, 32 * c),
                    )
                entry[1] += take
                budget -= take
                if entry[1] == 4:
                    pending.pop(0)
                    if tail_fn is not None:
                        tail_fn()

        for h in range(H_LOC):
            last_head = h == H_LOC - 1
            q_s, k_s, v_s, c_ps = loaded.pop(h)
            if not last_head:
                emit_loads(h + 1)

            # per-tile rowsum parts: [:, i, 0] and [:, i, 1] summed later
            rs_parts = st_pool.tile([P, NT, 2], f32, tag="rsp")
            nc.gpsimd.memset(rs_parts[:, :, :], 0.0)
            rowsum = st_pool.tile([P, NT], f32, tag="rowsum")
            g = st_pool.tile([P, NT], f32, tag="g")
            g_bf = st_pool.tile([P, NT], bf16, tag="gbf")
            e_tiles = []

            def make_tail(h=h, c_ps=c_ps, v_s=v_s, last_head=last_head):
                def tail():
                    # colsum [4 x 512 at partitions 0/32/64/96] -> csT [P, NT].
                    # m = 16p + t, so chunk-major cs4 [4x512] and csT [128,16]
                    # walk m in the same order: one direct SBUF->SBUF DMA.
                    cs4 = st_pool.tile([P, 512], f32, tag="cs4")
                    nc.vector.tensor_copy(cs4[:, :], c_ps[:, :])
                    csT = st_pool.tile([P, NT], f32, tag="csT")
                    nc.sync.dma_start(out=csT[:, :], in_=cs4[0:P:32, :])
                    o_s = vo_pool.tile([P, NT, D], f32, tag="o")
                    out_r = out[h].rearrange("(p t) d -> p t d", p=P)
                    if last_head:
                        # split the exposed tail across DVE + GpSimd so the
                        # multiply and the two stores overlap
                        parts = ((nc.vector, 0, 4), (nc.gpsimd, 4, NT))
                    else:
                        parts = ((nc.gpsimd, 0, NT),)
                    for eng, t0, t1 in parts:
                        eng.tensor_tensor(
                            o_s[:, t0:t1, :],
                            v_s[:, t0:t1, :],
                            csT[:, t0:t1].unsqueeze(-1).broadcast_to((P, t1 - t0, D)),
                            op=Mult,
                        )
                        nc.sync.dma_start(
                            out=out_r[:, t0:t1, :], in_=o_s[:, t0:t1, :]
                        )

                return tail

            tail_fn = make_tail()

            # g-batches: pairs, so g deliveries are fine-grained and the
            # colsum stream stays continuous; the last head finishes with
            # micro-batches so colsum work drains in-loop.
            dve_tiles = DVE_TILES_LAST if last_head else DVE_TILES
            batches = [(b0, b0 + 1) for b0 in range(0, NT - 2, 2)]
            if last_head:
                batches += [(14, 14), (15, 15)]
            else:
                batches += [(14, 15)]
            batch_of = {}
            for b0, b1 in batches:
                for j in range(b0, b1 + 1):
                    batch_of[j] = (b0, b1)

            # rowsum state per batch: slot at which the batch's parts are all
            # emitted (used to derive the colsum drip lag)
            dve_in_batch = {
                (b0, b1): [j for j in range(b0, b1 + 1) if j in dve_tiles]
                for b0, b1 in batches
            }

            for i in range(NT):
                slot += 1
                use_dve = i in dve_tiles
                e_i = e_pool.tile([P, N], bf16, tag="e")
                e_tiles.append(e_i)
                tmp = None
                if use_dve:
                    tmp = tmp_pool.tile([P, 2, 512], bf16, tag="t", name="tmp")
                for mh in range(MH):
                    s_ps = s_pool.tile([P, MW], f32, tag="s")
                    for c in range(MW // 512):
                        m0 = mh * MW + c * 512
                        nc.tensor.matmul(
                            s_ps[:, c * 512 : (c + 1) * 512],
                            lhsT=q_s[:, :, i * P : (i + 1) * P],
                            rhs=k_s[:, :, m0 : m0 + 512],
                            start=True,
                            stop=True,
                            perf_mode=DoubleRow,
                        )
                    if use_dve:
                        nc.vector.tensor_scalar(
                            e_i[:, mh * MW : (mh + 1) * MW].bitcast(i16),
                            s_ps[:, :],
                            SCH_A,
                            SCH_B,
                            op0=Mult,
                            op1=Add,
                        )
                        # GpSimd folds each half as soon as it lands, so the
                        # row-sum reduce can start right after the second exp
                        h0 = mh * MW
                        nc.gpsimd.tensor_tensor(
                            tmp[:, mh, :],
                            e_i[:, h0 : h0 + 512],
                            e_i[:, h0 + 512 : h0 + MW],
                            op=Add,
                        )
                    else:
                        nc.scalar.activation(
                            e_i[:, mh * MW : (mh + 1) * MW],
                            s_ps[:, :],
                            func=Exp,
                            scale=SCALE,
                            accum_out=rs_parts[:, i, mh : mh + 1],
                        )
                    # drip colsum chunk matmuls between the S-fill bursts so
                    # colsum work interleaves instead of delaying the fills
                    drip_colsum(2 if mh == 0 else (6 if len(pending) > 2 else 2))

                if use_dve:
                    def emit_reduce(i=i, tmp=tmp, rs_parts=rs_parts):
                        nc.vector.tensor_reduce(
                            rs_parts[:, i, 0:1],
                            tmp[:, :, :],
                            axis=mybir.AxisListType.XY,
                            op=Add,
                        )

                    red_pending.append((slot + 1, emit_reduce))

                # flush any due row-sum reduces before the batch boundary
                while red_pending and red_pending[0][0] <= slot:
                    red_pending.pop(0)[1]()

                if i == batch_of[i][1]:  # batch boundary: g for the batch
                    b0, b1 = batch_of[i]
                    # make sure every reduce this batch needs is emitted
                    while red_pending and red_pending[0][0] <= slot + 1:
                        red_pending.pop(0)[1]()
                    sl = slice(b0, b1 + 1)
                    nc.vector.tensor_tensor(
                        rowsum[:, sl],
                        rs_parts[:, sl, 0],
                        rs_parts[:, sl, 1],
                        op=Add,
                    )
                    nc.vector.reciprocal(g[:, sl], rowsum[:, sl])
                    nc.vector.tensor_copy(g_bf[:, sl], g[:, sl])
                    # colsum drip lag: conservative so the g semaphore has
                    # always fired by the time the PE reaches the colsum —
                    # a parked colsum head-of-line-blocks later S fills.  The
                    # head-final pair's g chain crosses the head boundary, so
                    # give it extra slack.
                    if b1 == NT - 1:
                        lag = 4
                    else:
                        lag = 3 if dve_in_batch[(b0, b1)] else 2
                    for idx, j in enumerate(range(b0, b1 + 1)):
                        pending.append(
                            [
                                slot + lag + idx,
                                0,
                                j,
                                c_ps,
                                g_bf,
                                e_tiles[j],
                                tail_fn if j == NT - 1 else None,
                            ]
                        )

            if last_head:
                while red_pending:
                    red_pending.pop(0)[1]()
                slot += NT  # make everything due
                drip_colsum(10**9)


_NC_CACHE = None


def _get_nc():
    global _NC_CACHE
    if _NC_CACHE is None:
        nc = bacc.Bacc("TRN2", target_bir_lowering=False, debug=False)
        qT = nc.dram_tensor(
            "qT", [H_LOC, D // 2, 2, N], f8e4, kind="ExternalInput"
        ).ap()
        kT = nc.dram_tensor(
            "kT", [H_LOC, D // 2, 2, N], f8e4, kind="ExternalInput"
        ).ap()
        vin = nc.dram_tensor("v", [H_LOC, N, D], f32, kind="ExternalInput").ap()
        out = nc.dram_tensor("out", [H_LOC, N, D], f32, kind="ExternalOutput").ap()
        with tile.TileContext(nc) as tc:
            _attention_kernel(tc, out, qT, kT, vin)
        nc.compile()
        _NC_CACHE = nc
    return _NC_CACHE


def kernel(q, k, v):
    q = np.asarray(q, dtype=np.float32).reshape(B * H, N, D)
    k = np.asarray(k, dtype=np.float32).reshape(B * H, N, D)
    v = np.asarray(v, dtype=np.float32).reshape(B * H, N, D)

    f8np = mybir.dt.np(f8e4)
    in_maps = []
    for c in range(N_CORES):
        sl = slice(H_LOC * c, H_LOC * (c + 1))
        qT8 = (
            np.ascontiguousarray(q[sl].transpose(0, 2, 1))
            .astype(f8np)
            .reshape(H_LOC, D // 2, 2, N)
        )
        kT8 = (
            np.ascontiguousarray(k[sl].transpose(0, 2, 1))
            .astype(f8np)
            .reshape(H_LOC, D // 2, 2, N)
        )
        in_maps.append({"qT": qT8, "kT": kT8, "v": np.ascontiguousarray(v[sl])})

    trace = bool(os.environ.get("KERNEL_TRACE"))
    res = run_bass_kernel_spmd(
        _get_nc(), in_maps, core_ids=list(range(N_CORES)), trace=trace
    )
    if trace:
        print(f"HW exec time: {res.exec_time_ns} ns")
        if res.instructions_and_trace is not None:
            print(f"trace: {res.instructions_and_trace[1]}")

    outs = [r["out"] for r in res.results]
    return np.concatenate(outs, axis=0).reshape(B, H, N, D)


# revision 58
# speedup vs baseline: 1.2490x; 1.0713x over previous
# Trainium2 Bass kernel for nn_MultiHeadAttention_48533130445634.
#
# Math (faithful to the reference, including its unusual second einsum):
#   scores[b,h,n,m] = softmax_m( (q[b,h,n,:] . k[b,h,m,:]) * 0.125 )
#   out[b,h,m,d]    = (sum_n scores[b,h,n,m]) * v[b,h,m,d]
#
# i.e. the output is V scaled elementwise by the column-sums of the softmax
# matrix.  Per (b,h), tiled over n (128 rows at a time):
#   S_i = Q_i K^T            (PE, fp32r, PSUM out, 1024-wide halves)
#   E_i = exp(S_i * 0.125)   split across two engines:
#     ACT tiles: scalar-engine exp (bf16 out to SBUF) with the ACT
#       accumulator emitting the per-half row-sum for free.
#     DVE tiles: Schraudolph-style exp on the vector engine -- one
#       tensor_scalar (S*A + B) written through an int16 bitcast of the bf16
#       E tile; the integer lands in the bf16 exponent/mantissa fields so the
#       bits ARE ~exp(S*0.125).  GpSimd adds the two m-halves, DVE reduces
#       the sum to the row-sum.  This offloads ~40% of the exp roofline off
#       the scalar engine.
#   g_i = 1 / rowsum_i       (DVE reciprocal, batched by 4 tiles)
#   colsum += g_i^T @ E_i    (PE, bf16; accumulated in one PSUM bank using
#                             four output base-partitions 0/32/64/96, one
#                             per 512-wide m-chunk)
#   out[m,d] = colsum[m] * v[m,d]   (GpSimd tensor_scalar)
#
# Colsum matmuls are queued and drip-fed (one tile's worth per n-tile slot,
# only once their g is ready) so they never head-of-line-block the S fills.
# The m index maps to partitions as m = 16p + t so V loads, output stores
# and the colsum scatter are all contiguous per partition.
#
# Sharding: 64 (b,h) pairs split across 8 cores, 8 pairs each (SPMD, no
# cross-core communication).  Q/K are pre-transposed on the host so the
# contraction dim (Dh=64) lands on SBUF partitions for the PE.

import math
import os

import numpy as np

import concourse.mybir as mybir
import concourse.tile as tile
from concourse import bacc
from concourse.bass_utils import run_bass_kernel_spmd

B, H, N, D = 4, 16, 2048, 64
N_CORES = 8
H_LOC = (B * H) // N_CORES  # 8 (b,h) pairs per core
P = 128                     # partition tile along n
NT = N // P                 # 16 n-tiles
SCALE = 0.125               # (DIM // N_HEADS) ** -0.5
MH = 2                      # m processed in halves of 1024 (PSUM bank budget)
MW = N // MH                # 1024

# Schraudolph exp for bf16: bitcast(int16(A*S + B)) ~= exp(S*SCALE).
SCH_A = SCALE * 128.0 / math.log(2.0)
SCH_B = 127.0 * 128.0 - 4.0

# n-tiles (within a head) whose exp runs on the vector engine via the
# Schraudolph trick; the rest run on the scalar engine with the ACT
# accumulator producing the row-sum.  Evenly interleaved, at g-pair starts
# (so the slower DVE row-sum path hides in slots where the DVE has no exp
# work), keeping ACT runs short; the last head keeps its final tiles on the
# fast ACT path so the tail drains quickly.
# Two ACT tiles between consecutive DVE tiles: the DVE's row-sum reduce and
# g bookkeeping then run in the slots where the DVE has no exp work, instead
# of delaying its next exp (which would stall the next S fill through the
# 3-buffer s_ps WAR rotation and starve the scalar engine).
DVE_TILES = frozenset(
    int(x) for x in os.environ.get("KERNEL_DVE_TILES", "0,3,6,9,12,14").split(",")
)
# the last head keeps its final tiles on the fast ACT-accumulator path so
# the exposed output tail drains as soon as possible
DVE_TILES_LAST = frozenset(
    int(x)
    for x in os.environ.get("KERNEL_DVE_TILES_LAST", "0,3,6,9,11").split(",")
)
FOLD3 = os.environ.get("KERNEL_FOLD3", "0") == "1"

f32 = mybir.dt.float32
f32r = mybir.dt.float32r
bf16 = mybir.dt.bfloat16
i16 = mybir.dt.int16
f8e4 = mybir.dt.float8e4
Exp = mybir.ActivationFunctionType.Exp
Add = mybir.AluOpType.add
Mult = mybir.AluOpType.mult
DoubleRow = mybir.MatmulPerfMode.DoubleRow


def _attention_kernel(tc, out, qT, kT, vin):
    nc = tc.nc

    with (
        tc.tile_pool(name="qk", bufs=3) as qk_pool,
        tc.tile_pool(name="ev", bufs=14) as e_pool,
        tc.tile_pool(name="vo", bufs=4) as vo_pool,
        tc.tile_pool(name="st", bufs=2) as st_pool,
        tc.tile_pool(name="tm", bufs=3) as tmp_pool,
        tc.tile_pool(name="s_ps", bufs=3, space="PSUM") as s_pool,
        tc.tile_pool(name="c_ps", bufs=2, space="PSUM") as c_pool,
    ):
        # Preload the exp table set and start the PE p-state ramp while the
        # first DMAs are in flight.
        warm = st_pool.tile([P, 1], f32, tag="warm")
        nc.gpsimd.memset(warm[:, :], 0.0)
        nc.scalar.activation(warm[:, :], warm[:, :], func=Exp)
        warm_ps = c_pool.tile([1, 1], f32, tag="csum")
        nc.tensor.matmul(
            warm_ps[0:1, 0:1], lhsT=warm[0:1, 0:1], rhs=warm[0:1, 0:1],
            start=True, stop=True, skip_group_check=True,
        )

        # Q/K/V loads for head h, emitted one head ahead so the SP sequencer
        # issues them before it blocks on the previous head's tail DMAs.
        loaded = {}

        def emit_loads(h, first=False):
            # q/k in fp8e4m3, laid out [32, 2, N]: contraction d = 2*dp + j
            # so the S matmul runs in DoubleRow mode (2 fp8 weights per PE
            # cell -> half the cycles per output row)
            q_s = qk_pool.tile([D // 2, 2, N], f8e4, tag="q")
            k_s = qk_pool.tile([D // 2, 2, N], f8e4, tag="k")
            if first:
                # order so the very first S fill's operands land earliest:
                # tile 0 needs all of k's first half but only q cols 0:256
                # (which also covers tile 1's lhsT)
                parts = [(k_s, kT, 0, MW), (q_s, qT, 0, 2 * P),
                         (q_s, qT, 2 * P, MW), (k_s, kT, MW, N),
                         (q_s, qT, MW, N)]
                for t_s, src, lo, hi in parts:
                    nc.sync.dma_start(
                        out=t_s[:, :, lo:hi], in_=src[h, :, :, lo:hi]
                    )
            else:
                for half in range(2):
                    sl = slice(half * MW, (half + 1) * MW)
                    nc.sync.dma_start(out=k_s[:, :, sl], in_=kT[h, :, :, sl])
                    nc.sync.dma_start(out=q_s[:, :, sl], in_=qT[h, :, :, sl])
            v_s = vo_pool.tile([P, NT, D], f32, tag="v")
            nc.sync.dma_start(
                out=v_s[:, :, :], in_=vin[h].rearrange("(p t) d -> p t d", p=P)
            )
            # the colsum accumulator needs no memset: the first matmul into
            # each bank-chunk has start=True (overwrite), and only the four
            # written rows are ever read back
            c_ps = c_pool.tile([P, 512], f32, tag="csum")
            loaded[h] = (q_s, k_s, v_s, c_ps)

        emit_loads(0, first=True)

        # colsum matmuls pending emission: [min_slot, chunks_done, j, c_ps,
        # g_bf, e_j, tail]
        pending = []
        # DVE-tile row-sum reduces pending emission: (min_slot, fn)
        red_pending = []
        slot = 0

        def drip_colsum(budget):
            """Emit up to `budget` pending colsum chunk matmuls whose due
            slot has arrived.  Chunk-granular so PE colsum work interleaves
            tightly between S fills instead of bursting ahead of them."""
            while budget > 0 and pending and pending[0][0] <= slot:
                entry = pending[0]
                _, done, j, c_ps, g_bf, e_j, tail_fn = entry
                take = min(budget, 4 - done)
                for c in range(done, done + take):
                    nc.tensor.matmul(
                        c_ps[32 * c : 32 * c + 1, :],
                        lhsT=g_bf[:, j : j + 1],
                        rhs=e_j[:, c * 512 : (c + 1) * 512],
                        start=(j == 0),
                        stop=(j == NT - 1),
                        skip_group_check=True,
                        tile_position=(0, 32 * c),
                    )
                entry[1] += take
                budget -= take
                if entry[1] == 4:
                    pending.pop(0)
                    if tail_fn is not None:
                        tail_fn()

        for h in range(H_LOC):
            last_head = h == H_LOC - 1
            q_s, k_s, v_s, c_ps = loaded.pop(h)
            if not last_head:
                emit_loads(h + 1)

            # per-tile rowsum parts for ACT tiles ([:, i, 0] + [:, i, 1],
            # summed into rowsum at the pair boundary); DVE tiles' reduces
            # write rowsum directly, so rs_parts never needs clearing (and
            # no memset to WAR against the next head's first accum).
            rs_parts = st_pool.tile([P, NT, 2], f32, tag="rsp")
            rowsum = st_pool.tile([P, NT], f32, tag="rowsum")
            g = st_pool.tile([P, NT], f32, tag="g")
            g_bf = st_pool.tile([P, NT], bf16, tag="gbf")
            e_tiles = []

            def make_tail(h=h, c_ps=c_ps, v_s=v_s, last_head=last_head):
                def tail():
                    # colsum [4 x 512 at partitions 0/32/64/96] -> csT [P, NT].
                    # m = 16p + t, so chunk-major cs4 [4x512] and csT [128,16]
                    # walk m in the same order: one direct SBUF->SBUF DMA.
                    cs4 = st_pool.tile([P, 512], f32, tag="cs4")
                    nc.vector.tensor_copy(cs4[:, :], c_ps[:, :])
                    csT = st_pool.tile([P, NT], f32, tag="csT")
                    nc.sync.dma_start(out=csT[:, :], in_=cs4[0:P:32, :])
                    o_s = vo_pool.tile([P, NT, D], f32, tag="o")
                    out_r = out[h].rearrange("(p t) d -> p t d", p=P)
                    if last_head:
                        # split the exposed tail across DVE + GpSimd so the
                        # multiply and the two stores overlap; the GpSimd
                        # share is smaller since it runs ~2.4x slower
                        parts = ((nc.vector, 0, 8), (nc.gpsimd, 8, NT))
                    else:
                        parts = ((nc.gpsimd, 0, NT),)
                    for eng, t0, t1 in parts:
                        eng.tensor_tensor(
                            o_s[:, t0:t1, :],
                            v_s[:, t0:t1, :],
                            csT[:, t0:t1].unsqueeze(-1).broadcast_to((P, t1 - t0, D)),
                            op=Mult,
                        )
                        nc.sync.dma_start(
                            out=out_r[:, t0:t1, :], in_=o_s[:, t0:t1, :]
                        )

                return tail

            tail_fn = make_tail()

            # g-batches: pairs, so g deliveries are fine-grained and the
            # colsum stream stays continuous; the last head finishes with
            # micro-batches so colsum work drains in-loop.
            dve_tiles = DVE_TILES_LAST if last_head else DVE_TILES
            batches = [(b0, b0 + 1) for b0 in range(0, NT - 2, 2)]
            if last_head:
                batches += [(14, 14), (15, 15)]
            else:
                batches += [(14, 15)]
            batch_of = {}
            for b0, b1 in batches:
                for j in range(b0, b1 + 1):
                    batch_of[j] = (b0, b1)

            # rowsum state per batch: slot at which the batch's parts are all
            # emitted (used to derive the colsum drip lag)
            dve_in_batch = {
                (b0, b1): [j for j in range(b0, b1 + 1) if j in dve_tiles]
                for b0, b1 in batches
            }

            for i in range(NT):
                slot += 1
                use_dve = i in dve_tiles
                e_i = e_pool.tile([P, N], bf16, tag="e")
                e_tiles.append(e_i)
                tmp = None
                if use_dve:
                    tmp = tmp_pool.tile([P, 2, 512], bf16, tag="t", name="tmp")
                for mh in range(MH):
                    s_ps = s_pool.tile([P, MW], f32, tag="s")
                    for c in range(MW // 512):
                        m0 = mh * MW + c * 512
                        nc.tensor.matmul(
                            s_ps[:, c * 512 : (c + 1) * 512],
                            lhsT=q_s[:, :, i * P : (i + 1) * P],
                            rhs=k_s[:, :, m0 : m0 + 512],
                            start=True,
                            stop=True,
                            perf_mode=DoubleRow,
                        )
                    if use_dve:
                        nc.vector.tensor_scalar(
                            e_i[:, mh * MW : (mh + 1) * MW].bitcast(i16),
                            s_ps[:, :],
                            SCH_A,
                            SCH_B,
                            op0=Mult,
                            op1=Add,
                        )
                        # GpSimd folds each half as soon as it lands, so the
                        # row-sum reduce can start right after the second exp
                        h0 = mh * MW
                        nc.gpsimd.tensor_tensor(
                            tmp[:, mh, :],
                            e_i[:, h0 : h0 + 512],
                            e_i[:, h0 + 512 : h0 + MW],
                            op=Add,
                        )
                    else:
                        nc.scalar.activation(
                            e_i[:, mh * MW : (mh + 1) * MW],
                            s_ps[:, :],
                            func=Exp,
                            scale=SCALE,
                            accum_out=rs_parts[:, i, mh : mh + 1],
                        )
                    # drip colsum chunk matmuls between the S-fill bursts so
                    # colsum work interleaves instead of delaying the fills
                    drip_colsum(2 if mh == 0 else (6 if len(pending) > 2 else 4))

                if use_dve:
                    if FOLD3:
                        # third GpSimd fold halves the DVE reduce width again
                        tmp2 = tmp_pool.tile([P, 512], bf16, tag="t2", name="t2")
                        nc.gpsimd.tensor_tensor(
                            tmp2[:, :], tmp[:, 0, :], tmp[:, 1, :], op=Add
                        )
                        red_src, red_ax = tmp2[:, :], mybir.AxisListType.X
                    else:
                        red_src, red_ax = tmp[:, :, :], mybir.AxisListType.XY

                    def emit_reduce(i=i, red_src=red_src, red_ax=red_ax,
                                    rowsum=rowsum):
                        nc.vector.tensor_reduce(
                            rowsum[:, i : i + 1],
                            red_src,
                            axis=red_ax,
                            op=Add,
                        )

                    red_pending.append(
                        (slot + int(os.environ.get("KERNEL_RED_LAG", "2")), emit_reduce)
                    )

                # flush any due row-sum reduces before the batch boundary
                while red_pending and red_pending[0][0] <= slot:
                    red_pending.pop(0)[1]()

                if i == batch_of[i][1]:  # batch boundary: g for the batch
                    b0, b1 = batch_of[i]
                    # make sure every reduce this batch needs is emitted
                    while red_pending and red_pending[0][0] <= slot + 1:
                        red_pending.pop(0)[1]()
                    sl = slice(b0, b1 + 1)
                    act_js = [j for j in (b0, b1) if j not in dve_tiles]
                    if act_js and act_js == list(range(act_js[0], b1 + 1)):
                        asl = slice(act_js[0], b1 + 1)
                        nc.vector.tensor_tensor(
                            rowsum[:, asl],
                            rs_parts[:, asl, 0],
                            rs_parts[:, asl, 1],
                            op=Add,
                        )
                    else:
                        for j in act_js:
                            nc.vector.tensor_tensor(
                                rowsum[:, j : j + 1],
                                rs_parts[:, j : j + 1, 0],
                                rs_parts[:, j : j + 1, 1],
                                op=Add,
                            )
                    nc.vector.reciprocal(g[:, sl], rowsum[:, sl])
                    nc.vector.tensor_copy(g_bf[:, sl], g[:, sl])
                    # colsum drip lag: a parked colsum head-of-line-blocks
                    # the S fills queued behind it, stalling the exp engines,
                    # so defer colsums far past the worst-case g chain — the
                    # PE has plenty of slack to absorb them late.  The last
                    # head drains sooner: once its fills are done, parking is
                    # harmless.
                    lag = 3 if last_head else 6
                    for idx, j in enumerate(range(b0, b1 + 1)):
                        pending.append(
                            [
                                slot + lag + idx,
                                0,
                                j,
                                c_ps,
                                g_bf,
                                e_tiles[j],
                                tail_fn if j == NT - 1 else None,
                            ]
                        )

            if last_head:
                while red_pending:
                    red_pending.pop(0)[1]()
                slot += NT  # make everything due
                drip_colsum(10**9)


_NC_CACHE = None


def _get_nc():
    global _NC_CACHE
    if _NC_CACHE is None:
        nc = bacc.Bacc("TRN2", target_bir_lowering=False, debug=False)
        qT = nc.dram_tensor(
            "qT", [H_LOC, D // 2, 2, N], f8e4, kind="ExternalInput"
        ).ap()
        kT = nc.dram_tensor(
            "kT", [H_LOC, D // 2, 2, N], f8e4, kind="ExternalInput"
        ).ap()
        vin = nc.dram_tensor("v", [H_LOC, N, D], f32, kind="ExternalInput").ap()
        out = nc.dram_tensor("out", [H_LOC, N, D], f32, kind="ExternalOutput").ap()
        with tile.TileContext(nc) as tc:
            _attention_kernel(tc, out, qT, kT, vin)
        nc.compile()
        _NC_CACHE = nc
    return _NC_CACHE


def kernel(q, k, v):
    q = np.asarray(q, dtype=np.float32).reshape(B * H, N, D)
    k = np.asarray(k, dtype=np.float32).reshape(B * H, N, D)
    v = np.asarray(v, dtype=np.float32).reshape(B * H, N, D)

    f8np = mybir.dt.np(f8e4)
    in_maps = []
    for c in range(N_CORES):
        sl = slice(H_LOC * c, H_LOC * (c + 1))
        qT8 = (
            np.ascontiguousarray(q[sl].transpose(0, 2, 1))
            .astype(f8np)
            .reshape(H_LOC, D // 2, 2, N)
        )
        kT8 = (
            np.ascontiguousarray(k[sl].transpose(0, 2, 1))
            .astype(f8np)
            .reshape(H_LOC, D // 2, 2, N)
        )
        in_maps.append({"qT": qT8, "kT": kT8, "v": np.ascontiguousarray(v[sl])})

    trace = bool(os.environ.get("KERNEL_TRACE"))
    res = run_bass_kernel_spmd(
        _get_nc(), in_maps, core_ids=list(range(N_CORES)), trace=trace
    )
    if trace:
        print(f"HW exec time: {res.exec_time_ns} ns")
        if res.instructions_and_trace is not None:
            print(f"trace: {res.instructions_and_trace[1]}")

    outs = [r["out"] for r in res.results]
    return np.concatenate(outs, axis=0).reshape(B, H, N, D)


# revision 63
# speedup vs baseline: 1.3424x; 1.0747x over previous
# Trainium2 Bass kernel for nn_MultiHeadAttention_48533130445634.
#
# Math (faithful to the reference, including its unusual second einsum):
#   scores[b,h,n,m] = softmax_m( (q[b,h,n,:] . k[b,h,m,:]) * 0.125 )
#   out[b,h,m,d]    = (sum_n scores[b,h,n,m]) * v[b,h,m,d]
#
# i.e. the output is V scaled elementwise by the column-sums of the softmax
# matrix.  Per (b,h), tiled over n (128 rows at a time):
#   S_i = Q_i K^T   (PE, q/k in fp8e4m3 with DoubleRow: the host splits the
#                    Dh=64 contraction into [32, 2, n] so two fp8 weights
#                    pack per PE cell -- half the cycles per output column)
#   E_i = exp(S_i * 0.125)   split across two engines:
#     ACT tiles: scalar-engine exp (bf16 out to SBUF) with the ACT
#       accumulator emitting the per-half row-sum for free.
#     DVE tiles: Schraudolph-style exp on the vector engine -- one
#       tensor_scalar (S*A + B) written through an int16 bitcast of the bf16
#       E tile; the integer lands in the bf16 exponent/mantissa fields so the
#       bits ARE ~exp(S*0.125).  GpSimd adds the m-halves pairwise, DVE
#       reduces the folded half to the row-sum.  This offloads ~40% of the
#       exp roofline off the scalar engine (the overall bottleneck).
#   g_i = 1 / rowsum_i       (DVE reciprocal, per pair of tiles)
#   colsum += g_i^T @ E_i    (PE, bf16; accumulated in one PSUM bank using
#                             four output base-partitions 0/32/64/96, one
#                             per 512-wide m-chunk)
#   out[m,d] = colsum[m] * v[m,d]   (GpSimd tensor_tensor)
#
# Scheduling notes (what the structure below is for):
#   - ACT is the critical engine (~200us busy); everything else must never
#     stall it.  Its exp pipeline is kept self-paced by giving ACT tiles a
#     dedicated 2-buffer PSUM S pool and DVE tiles their own 1-buffer pool,
#     so the PSUM WAR chain never threads an ACT exp behind a DVE exp.
#   - A DVE tile's second m-half is deferred one slot: its S fill WARs the
#     first half's exp (single buffer) and would otherwise park at the head
#     of the PE queue, blocking later fills (the PE wait queue is in-order).
#   - Colsum matmuls wait on g; a parked colsum blocks the S fills queued
#     behind it, so they are drip-fed chunk-wise with a generous lag (the PE
#     has ~60us slack) and g-pair boundaries are emitted event-driven, even
#     when a pair's row-sum chain crosses into the next head's slots.
#   - The m index maps to partitions as m = 16p + t so V loads, output
#     stores and the colsum scatter are all contiguous per partition.
#
# Sharding: 64 (b,h) pairs split across 8 cores, 8 pairs each (SPMD, no
# cross-core communication).  Q/K are pre-transposed on the host so the
# contraction dim lands on SBUF partitions for the PE.

import math
import os

import numpy as np

import concourse.mybir as mybir
import concourse.tile as tile
from concourse import bacc
from concourse.bass_utils import run_bass_kernel_spmd

B, H, N, D = 4, 16, 2048, 64
N_CORES = 8
H_LOC = (B * H) // N_CORES  # 8 (b,h) pairs per core
P = 128                     # partition tile along n
NT = N // P                 # 16 n-tiles
SCALE = 0.125               # (DIM // N_HEADS) ** -0.5
MH = 2                      # m processed in halves of 1024 (PSUM bank budget)
MW = N // MH                # 1024

# Schraudolph exp for bf16: bitcast(int16(A*S + B)) ~= exp(S*SCALE).
SCH_A = SCALE * 128.0 / math.log(2.0)
SCH_B = 127.0 * 128.0 - 4.0

# n-tiles (within a head) whose exp runs on the vector engine via the
# Schraudolph trick; the rest run on the scalar engine with the ACT
# accumulator producing the row-sum.  Evenly interleaved, at g-pair starts
# (so the slower DVE row-sum path hides in slots where the DVE has no exp
# work), keeping ACT runs short; the last head keeps its final tiles on the
# fast ACT path so the tail drains quickly.
# Two ACT tiles between consecutive DVE tiles: the DVE's row-sum reduce and
# g bookkeeping then run in the slots where the DVE has no exp work, instead
# of delaying its next exp (which would stall the next S fill through the
# 3-buffer s_ps WAR rotation and starve the scalar engine).
DVE_TILES = frozenset(
    int(x) for x in os.environ.get("KERNEL_DVE_TILES", "0,3,6,9,12,14").split(",")
)
# the last head keeps its final tiles on the fast ACT-accumulator path so
# the exposed output tail drains as soon as possible
DVE_TILES_LAST = frozenset(
    int(x)
    for x in os.environ.get("KERNEL_DVE_TILES_LAST", "0,3,6,9,11").split(",")
)
FOLD3 = os.environ.get("KERNEL_FOLD3", "0") == "1"

f32 = mybir.dt.float32
f32r = mybir.dt.float32r
bf16 = mybir.dt.bfloat16
i16 = mybir.dt.int16
f8e4 = mybir.dt.float8e4
Exp = mybir.ActivationFunctionType.Exp
Add = mybir.AluOpType.add
Mult = mybir.AluOpType.mult
DoubleRow = mybir.MatmulPerfMode.DoubleRow


def _attention_kernel(tc, out, qT, kT, vin):
    nc = tc.nc

    with (
        tc.tile_pool(name="qk", bufs=3) as qk_pool,
        tc.tile_pool(name="ev", bufs=14) as e_pool,
        tc.tile_pool(name="vo", bufs=4) as vo_pool,
        tc.tile_pool(name="st", bufs=2) as st_pool,
        tc.tile_pool(name="tm", bufs=3) as tmp_pool,
        tc.tile_pool(name="sa_ps", bufs=2, space="PSUM") as sa_pool,
        tc.tile_pool(name="sd_ps", bufs=1, space="PSUM") as sd_pool,
        tc.tile_pool(name="c_ps", bufs=2, space="PSUM") as c_pool,
    ):
        # Preload the exp table set and start the PE p-state ramp while the
        # first DMAs are in flight.
        warm = st_pool.tile([P, 1], f32, tag="warm")
        nc.gpsimd.memset(warm[:, :], 0.0)
        nc.scalar.activation(warm[:, :], warm[:, :], func=Exp)
        warm_ps = c_pool.tile([1, 1], f32, tag="csum")
        nc.tensor.matmul(
            warm_ps[0:1, 0:1], lhsT=warm[0:1, 0:1], rhs=warm[0:1, 0:1],
            start=True, stop=True, skip_group_check=True,
        )

        # Q/K/V loads for head h, emitted one head ahead so the SP sequencer
        # issues them before it blocks on the previous head's tail DMAs.
        loaded = {}

        def emit_loads(h, first=False):
            # q/k in fp8e4m3, laid out [32, 2, N]: contraction d = 2*dp + j
            # so the S matmul runs in DoubleRow mode (2 fp8 weights per PE
            # cell -> half the cycles per output row)
            q_s = qk_pool.tile([D // 2, 2, N], f8e4, tag="q")
            k_s = qk_pool.tile([D // 2, 2, N], f8e4, tag="k")
            if first:
                # order so the very first S fill's operands land earliest:
                # tile 0 needs all of k's first half but only q cols 0:256
                # (which also covers tile 1's lhsT)
                parts = [(k_s, kT, 0, MW), (q_s, qT, 0, 2 * P),
                         (q_s, qT, 2 * P, MW), (k_s, kT, MW, N),
                         (q_s, qT, MW, N)]
                for t_s, src, lo, hi in parts:
                    nc.sync.dma_start(
                        out=t_s[:, :, lo:hi], in_=src[h, :, :, lo:hi]
                    )
            else:
                for half in range(2):
                    sl = slice(half * MW, (half + 1) * MW)
                    nc.sync.dma_start(out=k_s[:, :, sl], in_=kT[h, :, :, sl])
                    nc.sync.dma_start(out=q_s[:, :, sl], in_=qT[h, :, :, sl])
            v_s = vo_pool.tile([P, NT, D], f32, tag="v")
            nc.sync.dma_start(
                out=v_s[:, :, :], in_=vin[h].rearrange("(p t) d -> p t d", p=P)
            )
            # the colsum accumulator needs no memset: the first matmul into
            # each bank-chunk has start=True (overwrite), and only the four
            # written rows are ever read back
            c_ps = c_pool.tile([P, 512], f32, tag="csum")
            loaded[h] = (q_s, k_s, v_s, c_ps)

        emit_loads(0, first=True)

        # colsum matmuls pending emission: [min_slot, chunks_done, j, c_ps,
        # g_bf, e_j, tail]
        pending = []
        # DVE-tile row-sum reduces pending emission: (min_slot, fn)
        red_pending = []
        # g-pair boundaries not yet emitted: (ready_fn, emit_fn) in order.
        # Outer-scope: a pair whose row-sum chain crosses a head boundary
        # (e.g. a DVE tile at position 14) must still be flushed during the
        # next head's slots.
        bnd_pending = []
        slot = 0

        def flush_boundaries():
            while bnd_pending and bnd_pending[0][0]():
                bnd_pending.pop(0)[1]()

        def drip_colsum(budget):
            """Emit up to `budget` pending colsum chunk matmuls whose due
            slot has arrived.  Chunk-granular so PE colsum work interleaves
            tightly between S fills instead of bursting ahead of them."""
            while budget > 0 and pending and pending[0][0] <= slot:
                entry = pending[0]
                _, done, j, c_ps, g_bf, e_j, tail_fn = entry
                take = min(budget, 4 - done)
                for c in range(done, done + take):
                    nc.tensor.matmul(
                        c_ps[32 * c : 32 * c + 1, :],
                        lhsT=g_bf[:, j : j + 1],
                        rhs=e_j[:, c * 512 : (c + 1) * 512],
                        start=(j == 0),
                        stop=(j == NT - 1),
                        skip_group_check=True,
                        tile_position=(0, 32 * c),
                    )
                entry[1] += take
                budget -= take
                if entry[1] == 4:
                    pending.pop(0)
                    if tail_fn is not None:
                        tail_fn()

        for h in range(H_LOC):
            last_head = h == H_LOC - 1
            q_s, k_s, v_s, c_ps = loaded.pop(h)
            if not last_head:
                emit_loads(h + 1)

            # per-tile rowsum parts for ACT tiles ([:, i, 0] + [:, i, 1],
            # summed into rowsum at the pair boundary); DVE tiles' reduces
            # write rowsum directly, so rs_parts never needs clearing (and
            # no memset to WAR against the next head's first accum).
            rs_parts = st_pool.tile([P, NT, 2], f32, tag="rsp")
            rowsum = st_pool.tile([P, NT], f32, tag="rowsum")
            g = st_pool.tile([P, NT], f32, tag="g")
            g_bf = st_pool.tile([P, NT], bf16, tag="gbf")
            e_tiles = []

            def make_tail(h=h, c_ps=c_ps, v_s=v_s, last_head=last_head):
                def tail():
                    # colsum [4 x 512 at partitions 0/32/64/96] -> csT [P, NT].
                    # m = 16p + t, so chunk-major cs4 [4x512] and csT [128,16]
                    # walk m in the same order: one direct SBUF->SBUF DMA.
                    cs4 = st_pool.tile([P, 512], f32, tag="cs4")
                    nc.vector.tensor_copy(cs4[:, :], c_ps[:, :])
                    csT = st_pool.tile([P, NT], f32, tag="csT")
                    nc.sync.dma_start(out=csT[:, :], in_=cs4[0:P:32, :])
                    o_s = vo_pool.tile([P, NT, D], f32, tag="o")
                    out_r = out[h].rearrange("(p t) d -> p t d", p=P)
                    if last_head:
                        # split the exposed tail across DVE + GpSimd so the
                        # multiply and the two stores overlap; the GpSimd
                        # share is smaller since it runs ~2.4x slower
                        parts = ((nc.vector, 0, 8), (nc.gpsimd, 8, NT))
                    else:
                        parts = ((nc.gpsimd, 0, NT),)
                    for eng, t0, t1 in parts:
                        eng.tensor_tensor(
                            o_s[:, t0:t1, :],
                            v_s[:, t0:t1, :],
                            csT[:, t0:t1].unsqueeze(-1).broadcast_to((P, t1 - t0, D)),
                            op=Mult,
                        )
                        nc.sync.dma_start(
                            out=out_r[:, t0:t1, :], in_=o_s[:, t0:t1, :]
                        )

                return tail

            tail_fn = make_tail()

            # g-batches: pairs, so g deliveries are fine-grained and the
            # colsum stream stays continuous; the last head finishes with
            # micro-batches so colsum work drains in-loop.
            dve_tiles = DVE_TILES_LAST if last_head else DVE_TILES
            batches = [(b0, b0 + 1) for b0 in range(0, NT - 2, 2)]
            if last_head:
                batches += [(14, 14), (15, 15)]
            else:
                batches += [(14, 15)]
            batch_of = {}
            for b0, b1 in batches:
                for j in range(b0, b1 + 1):
                    batch_of[j] = (b0, b1)

            # rowsum state per batch: slot at which the batch's parts are all
            # emitted (used to derive the colsum drip lag)
            dve_in_batch = {
                (b0, b1): [j for j in range(b0, b1 + 1) if j in dve_tiles]
                for b0, b1 in batches
            }

            # --- per-tile emission machinery -------------------------
            # DVE tiles get a dedicated single S buffer (sd_pool) so the
            # ACT exp relay never threads through a DVE exp via PSUM WAR;
            # the DVE tile's second m-half is deferred one slot so its fill
            # (which WARs the first half's exp) never parks the PE queue.
            halves_emitted = {}
            reds_emitted = set()

            def emit_half(i, mh, e_i, tmp, use_dve):
                pool = sd_pool if use_dve else sa_pool
                s_ps = pool.tile([P, MW], f32, tag="sd" if use_dve else "sa",
                                 name="s_ps")
                for c in range(MW // 512):
                    m0 = mh * MW + c * 512
                    nc.tensor.matmul(
                        s_ps[:, c * 512 : (c + 1) * 512],
                        lhsT=q_s[:, :, i * P : (i + 1) * P],
                        rhs=k_s[:, :, m0 : m0 + 512],
                        start=True,
                        stop=True,
                        perf_mode=DoubleRow,
                    )
                if use_dve:
                    nc.vector.tensor_scalar(
                        e_i[:, mh * MW : (mh + 1) * MW].bitcast(i16),
                        s_ps[:, :],
                        SCH_A,
                        SCH_B,
                        op0=Mult,
                        op1=Add,
                    )
                    # GpSimd folds each half as soon as it lands, so the
                    # row-sum reduce is over 1024 elements, not 2048
                    h0 = mh * MW
                    nc.gpsimd.tensor_tensor(
                        tmp[:, mh, :],
                        e_i[:, h0 : h0 + 512],
                        e_i[:, h0 + 512 : h0 + MW],
                        op=Add,
                    )
                else:
                    nc.scalar.activation(
                        e_i[:, mh * MW : (mh + 1) * MW],
                        s_ps[:, :],
                        func=Exp,
                        scale=SCALE,
                        accum_out=rs_parts[:, i, mh : mh + 1],
                    )
                halves_emitted[i] = halves_emitted.get(i, 0) + 1
                if use_dve and mh == 1:
                    def emit_reduce(i=i, tmp=tmp, rowsum=rowsum,
                                    reds_emitted=reds_emitted):
                        nc.vector.tensor_reduce(
                            rowsum[:, i : i + 1],
                            tmp[:, :, :],
                            axis=mybir.AxisListType.XY,
                            op=Add,
                        )
                        reds_emitted.add(i)

                    red_pending.append((slot + 2, emit_reduce))

            # bind per-head state as defaults: the head loop shares one
            # Python scope, and a boundary can flush during the next head
            def emit_boundary(b0, b1, rowsum=rowsum, rs_parts=rs_parts, g=g,
                              g_bf=g_bf, dve_tiles=dve_tiles, c_ps=c_ps,
                              e_tiles=e_tiles, tail_fn=tail_fn,
                              last_head=last_head):
                sl = slice(b0, b1 + 1)
                act_js = [j for j in range(b0, b1 + 1) if j not in dve_tiles]
                if act_js and act_js == list(range(act_js[0], b1 + 1)):
                    asl = slice(act_js[0], b1 + 1)
                    nc.vector.tensor_tensor(
                        rowsum[:, asl],
                        rs_parts[:, asl, 0],
                        rs_parts[:, asl, 1],
                        op=Add,
                    )
                else:
                    for j in act_js:
                        nc.vector.tensor_tensor(
                            rowsum[:, j : j + 1],
                            rs_parts[:, j : j + 1, 0],
                            rs_parts[:, j : j + 1, 1],
                            op=Add,
                        )
                nc.vector.reciprocal(g[:, sl], rowsum[:, sl])
                nc.vector.tensor_copy(g_bf[:, sl], g[:, sl])
                # colsum drip lag: a parked colsum head-of-line-blocks the S
                # fills queued behind it, so defer far past the g chain; the
                # last head drains sooner (parking is harmless once its own
                # fills are done).
                lag = 3 if last_head else 5
                for idx, j in enumerate(range(b0, b1 + 1)):
                    pending.append(
                        [
                            slot + lag + idx,
                            0,
                            j,
                            c_ps,
                            g_bf,
                            e_tiles[j],
                            tail_fn if j == NT - 1 else None,
                        ]
                    )

            def make_ready(b0, b1, halves_emitted=halves_emitted,
                           reds_emitted=reds_emitted, dve_tiles=dve_tiles):
                def ready():
                    if any(halves_emitted.get(j, 0) < 2
                           for j in range(b0, b1 + 1)):
                        return False
                    return not any(j in dve_tiles and j not in reds_emitted
                                   for j in range(b0, b1 + 1))

                return ready

            for _b0, _b1 in batches:
                bnd_pending.append(
                    (make_ready(_b0, _b1),
                     lambda b0=_b0, b1=_b1, f=emit_boundary: f(b0, b1))
                )

            deferred_mh1 = []

            for i in range(NT):
                slot += 1
                use_dve = i in dve_tiles
                e_i = e_pool.tile([P, N], bf16, tag="e")
                e_tiles.append(e_i)
                tmp = None
                if use_dve:
                    tmp = tmp_pool.tile([P, 2, 512], bf16, tag="t", name="tmp")

                emit_half(i, 0, e_i, tmp, use_dve)
                drip_colsum(2)
                # the previous DVE tile's deferred second half
                while deferred_mh1:
                    deferred_mh1.pop(0)()
                if use_dve and i < NT - 1:
                    deferred_mh1.append(
                        lambda i=i, e_i=e_i, tmp=tmp: emit_half(i, 1, e_i, tmp, True)
                    )
                else:
                    emit_half(i, 1, e_i, tmp, use_dve)
                drip_colsum(6 if len(pending) > 2 else 4)

                # flush due row-sum reduces, then any completed g-pairs
                while red_pending and red_pending[0][0] <= slot:
                    red_pending.pop(0)[1]()
                flush_boundaries()

            # end of head: drain this head's remaining bookkeeping next head
            # (closures capture this head's tiles); the last head forces it
            if last_head:
                while deferred_mh1:
                    deferred_mh1.pop(0)()
                while red_pending:
                    red_pending.pop(0)[1]()
                flush_boundaries()
                assert not bnd_pending, "unflushed g-pair boundaries"

            if last_head:
                while red_pending:
                    red_pending.pop(0)[1]()
                slot += NT  # make everything due
                drip_colsum(10**9)


_NC_CACHE = None


def _get_nc():
    global _NC_CACHE
    if _NC_CACHE is None:
        nc = bacc.Bacc("TRN2", target_bir_lowering=False, debug=False)
        qT = nc.dram_tensor(
            "qT", [H_LOC, D // 2, 2, N], f8e4, kind="ExternalInput"
        ).ap()
        kT = nc.dram_tensor(
            "kT", [H_LOC, D // 2, 2, N], f8e4, kind="ExternalInput"
        ).ap()
        vin = nc.dram_tensor("v", [H_LOC, N, D], f32, kind="ExternalInput").ap()
        out = nc.dram_tensor("out", [H_LOC, N, D], f32, kind="ExternalOutput").ap()
        with tile.TileContext(nc) as tc:
            _attention_kernel(tc, out, qT, kT, vin)
        nc.compile()
        _NC_CACHE = nc
    return _NC_CACHE


def kernel(q, k, v):
    q = np.asarray(q, dtype=np.float32).reshape(B * H, N, D)
    k = np.asarray(k, dtype=np.float32).reshape(B * H, N, D)
    v = np.asarray(v, dtype=np.float32).reshape(B * H, N, D)

    f8np = mybir.dt.np(f8e4)
    in_maps = []
    for c in range(N_CORES):
        sl = slice(H_LOC * c, H_LOC * (c + 1))
        qT8 = (
            np.ascontiguousarray(q[sl].transpose(0, 2, 1))
            .astype(f8np)
            .reshape(H_LOC, D // 2, 2, N)
        )
        kT8 = (
            np.ascontiguousarray(k[sl].transpose(0, 2, 1))
            .astype(f8np)
            .reshape(H_LOC, D // 2, 2, N)
        )
        in_maps.append({"qT": qT8, "kT": kT8, "v": np.ascontiguousarray(v[sl])})

    trace = bool(os.environ.get("KERNEL_TRACE"))
    res = run_bass_kernel_spmd(
        _get_nc(), in_maps, core_ids=list(range(N_CORES)), trace=trace
    )
    if trace:
        print(f"HW exec time: {res.exec_time_ns} ns")
        if res.instructions_and_trace is not None:
            print(f"trace: {res.instructions_and_trace[1]}")

    outs = [r["out"] for r in res.results]
    return np.concatenate(outs, axis=0).reshape(B, H, N, D)


# revision 67
# speedup vs baseline: 1.3525x; 1.0075x over previous
# Trainium2 Bass kernel for nn_MultiHeadAttention_48533130445634.
#
# Math (faithful to the reference, including its unusual second einsum):
#   scores[b,h,n,m] = softmax_m( (q[b,h,n,:] . k[b,h,m,:]) * 0.125 )
#   out[b,h,m,d]    = (sum_n scores[b,h,n,m]) * v[b,h,m,d]
#
# i.e. the output is V scaled elementwise by the column-sums of the softmax
# matrix.  Per (b,h), tiled over n (128 rows at a time):
#   S_i = Q_i K^T   (PE, q/k in fp8e4m3 with DoubleRow: the host splits the
#                    Dh=64 contraction into [32, 2, n] so two fp8 weights
#                    pack per PE cell -- half the cycles per output column)
#   E_i = exp(S_i * 0.125)   split across two engines:
#     ACT tiles: scalar-engine exp (bf16 out to SBUF) with the ACT
#       accumulator emitting the per-half row-sum for free.
#     DVE tiles: Schraudolph-style exp on the vector engine -- one
#       tensor_scalar (S*A + B) written through an int16 bitcast of the bf16
#       E tile; the integer lands in the bf16 exponent/mantissa fields so the
#       bits ARE ~exp(S*0.125).  GpSimd adds the m-halves pairwise, DVE
#       reduces the folded half to the row-sum.  This offloads ~40% of the
#       exp roofline off the scalar engine (the overall bottleneck).
#   g_i = 1 / rowsum_i       (DVE reciprocal, per pair of tiles)
#   colsum += g_i^T @ E_i    (PE, bf16; accumulated in one PSUM bank using
#                             four output base-partitions 0/32/64/96, one
#                             per 512-wide m-chunk)
#   out[m,d] = colsum[m] * v[m,d]   (GpSimd tensor_tensor)
#
# Scheduling notes (what the structure below is for):
#   - ACT is the critical engine (~200us busy); everything else must never
#     stall it.  Its exp pipeline is kept self-paced by giving ACT tiles a
#     dedicated 2-buffer PSUM S pool and DVE tiles their own 1-buffer pool,
#     so the PSUM WAR chain never threads an ACT exp behind a DVE exp.
#   - A DVE tile's second m-half is deferred one slot: its S fill WARs the
#     first half's exp (single buffer) and would otherwise park at the head
#     of the PE queue, blocking later fills (the PE wait queue is in-order).
#   - Colsum matmuls wait on g; a parked colsum blocks the S fills queued
#     behind it, so they are drip-fed chunk-wise with a generous lag (the PE
#     has ~60us slack) and g-pair boundaries are emitted event-driven, even
#     when a pair's row-sum chain crosses into the next head's slots.
#   - The m index maps to partitions as m = 16p + t so V loads, output
#     stores and the colsum scatter are all contiguous per partition.
#
# Sharding: 64 (b,h) pairs split across 8 cores, 8 pairs each (SPMD, no
# cross-core communication).  Q/K are pre-transposed on the host so the
# contraction dim lands on SBUF partitions for the PE.

import math
import os

import numpy as np

import concourse.mybir as mybir
import concourse.tile as tile
from concourse import bacc
from concourse.bass_utils import run_bass_kernel_spmd

B, H, N, D = 4, 16, 2048, 64
N_CORES = 8
H_LOC = (B * H) // N_CORES  # 8 (b,h) pairs per core
P = 128                     # partition tile along n
NT = N // P                 # 16 n-tiles
SCALE = 0.125               # (DIM // N_HEADS) ** -0.5
MH = 2                      # m processed in halves of 1024 (PSUM bank budget)
MW = N // MH                # 1024

# Schraudolph exp for bf16: bitcast(int16(A*S + B)) ~= exp(S*SCALE).
SCH_A = SCALE * 128.0 / math.log(2.0)
SCH_B = 127.0 * 128.0 - 4.0

# n-tiles (within a head) whose exp runs on the vector engine via the
# Schraudolph trick; the rest run on the scalar engine with the ACT
# accumulator producing the row-sum.  Evenly interleaved, at g-pair starts
# (so the slower DVE row-sum path hides in slots where the DVE has no exp
# work), keeping ACT runs short; the last head keeps its final tiles on the
# fast ACT path so the tail drains quickly.
# Two ACT tiles between consecutive DVE tiles: the DVE's row-sum reduce and
# g bookkeeping then run in the slots where the DVE has no exp work, instead
# of delaying its next exp (which would stall the next S fill through the
# 3-buffer s_ps WAR rotation and starve the scalar engine).
DVE_TILES = frozenset(
    int(x) for x in os.environ.get("KERNEL_DVE_TILES", "0,3,6,9,12,14").split(",")
)
# the last head keeps its final tiles on the fast ACT-accumulator path so
# the exposed output tail drains as soon as possible
DVE_TILES_LAST = frozenset(
    int(x)
    for x in os.environ.get("KERNEL_DVE_TILES_LAST", "0,3,6,9,11").split(",")
)
FOLD3 = os.environ.get("KERNEL_FOLD3", "0") == "1"
DRIP0 = int(os.environ.get("KERNEL_DRIP0", "2"))
DRIP1 = int(os.environ.get("KERNEL_DRIP1", "4"))
# tiles whose first m-half runs on ACT but second on the DVE (uses the DVE
# S pool, so it never threads into the ACT exp relay) — shifts ~5% of the
# exp roofline off the critical scalar engine
MIXED_TILES = frozenset(
    int(x) for x in os.environ.get("KERNEL_MIXED_TILES", "4").split(",") if x
)

f32 = mybir.dt.float32
f32r = mybir.dt.float32r
bf16 = mybir.dt.bfloat16
i16 = mybir.dt.int16
f8e4 = mybir.dt.float8e4
Exp = mybir.ActivationFunctionType.Exp
Add = mybir.AluOpType.add
Mult = mybir.AluOpType.mult
DoubleRow = mybir.MatmulPerfMode.DoubleRow


def _attention_kernel(tc, out, qT, kT, vin):
    nc = tc.nc

    with (
        tc.tile_pool(name="qk", bufs=3) as qk_pool,
        tc.tile_pool(name="ev", bufs=14) as e_pool,
        tc.tile_pool(name="vo", bufs=4) as vo_pool,
        tc.tile_pool(name="st", bufs=2) as st_pool,
        tc.tile_pool(name="tm", bufs=3) as tmp_pool,
        tc.tile_pool(name="sa_ps", bufs=2, space="PSUM") as sa_pool,
        tc.tile_pool(name="sd_ps", bufs=1, space="PSUM") as sd_pool,
        tc.tile_pool(name="c_ps", bufs=2, space="PSUM") as c_pool,
    ):
        # Preload the exp table set and start the PE p-state ramp while the
        # first DMAs are in flight.
        warm = st_pool.tile([P, 1], f32, tag="warm")
        nc.gpsimd.memset(warm[:, :], 0.0)
        nc.scalar.activation(warm[:, :], warm[:, :], func=Exp)
        warm_ps = c_pool.tile([1, 1], f32, tag="csum")
        nc.tensor.matmul(
            warm_ps[0:1, 0:1], lhsT=warm[0:1, 0:1], rhs=warm[0:1, 0:1],
            start=True, stop=True, skip_group_check=True,
        )

        # Q/K/V loads for head h, emitted one head ahead so the SP sequencer
        # issues them before it blocks on the previous head's tail DMAs.
        loaded = {}

        def emit_loads(h, first=False):
            # q/k in fp8e4m3, laid out [32, 2, N]: contraction d = 2*dp + j
            # so the S matmul runs in DoubleRow mode (2 fp8 weights per PE
            # cell -> half the cycles per output row)
            q_s = qk_pool.tile([D // 2, 2, N], f8e4, tag="q")
            k_s = qk_pool.tile([D // 2, 2, N], f8e4, tag="k")
            if first:
                # order so the very first S fill's operands land earliest:
                # tile 0 needs all of k's first half but only q cols 0:256
                # (which also covers tile 1's lhsT)
                parts = [(k_s, kT, 0, MW), (q_s, qT, 0, 2 * P),
                         (q_s, qT, 2 * P, MW), (k_s, kT, MW, N),
                         (q_s, qT, MW, N)]
                for t_s, src, lo, hi in parts:
                    nc.sync.dma_start(
                        out=t_s[:, :, lo:hi], in_=src[h, :, :, lo:hi]
                    )
            else:
                for half in range(2):
                    sl = slice(half * MW, (half + 1) * MW)
                    nc.sync.dma_start(out=k_s[:, :, sl], in_=kT[h, :, :, sl])
                    nc.sync.dma_start(out=q_s[:, :, sl], in_=qT[h, :, :, sl])
            v_s = vo_pool.tile([P, NT, D], f32, tag="v")
            nc.sync.dma_start(
                out=v_s[:, :, :], in_=vin[h].rearrange("(p t) d -> p t d", p=P)
            )
            # the colsum accumulator needs no memset: the first matmul into
            # each bank-chunk has start=True (overwrite), and only the four
            # written rows are ever read back
            c_ps = c_pool.tile([P, 512], f32, tag="csum")
            loaded[h] = (q_s, k_s, v_s, c_ps)

        emit_loads(0, first=True)

        # colsum matmuls pending emission: [min_slot, chunks_done, j, c_ps,
        # g_bf, e_j, tail]
        pending = []
        # DVE-tile row-sum reduces pending emission: (min_slot, fn)
        red_pending = []
        # g-pair boundaries not yet emitted: (ready_fn, emit_fn) in order.
        # Outer-scope: a pair whose row-sum chain crosses a head boundary
        # (e.g. a DVE tile at position 14) must still be flushed during the
        # next head's slots.
        bnd_pending = []
        slot = 0

        def flush_boundaries():
            while bnd_pending and bnd_pending[0][0]():
                bnd_pending.pop(0)[1]()

        def drip_colsum(budget):
            """Emit up to `budget` pending colsum chunk matmuls whose due
            slot has arrived.  Chunk-granular so PE colsum work interleaves
            tightly between S fills instead of bursting ahead of them."""
            while budget > 0 and pending and pending[0][0] <= slot:
                entry = pending[0]
                _, done, j, c_ps, g_bf, e_j, tail_fn = entry
                take = min(budget, 4 - done)
                for c in range(done, done + take):
                    nc.tensor.matmul(
                        c_ps[32 * c : 32 * c + 1, :],
                        lhsT=g_bf[:, j : j + 1],
                        rhs=e_j[:, c * 512 : (c + 1) * 512],
                        start=(j == 0),
                        stop=(j == NT - 1),
                        skip_group_check=True,
                        tile_position=(0, 32 * c),
                    )
                entry[1] += take
                budget -= take
                if entry[1] == 4:
                    pending.pop(0)
                    if tail_fn is not None:
                        tail_fn()

        for h in range(H_LOC):
            last_head = h == H_LOC - 1
            q_s, k_s, v_s, c_ps = loaded.pop(h)
            if not last_head:
                emit_loads(h + 1)

            # per-tile rowsum parts for ACT tiles ([:, i, 0] + [:, i, 1],
            # summed into rowsum at the pair boundary); DVE tiles' reduces
            # write rowsum directly, so rs_parts never needs clearing (and
            # no memset to WAR against the next head's first accum).
            rs_parts = st_pool.tile([P, NT, 2], f32, tag="rsp")
            rowsum = st_pool.tile([P, NT], f32, tag="rowsum")
            g = st_pool.tile([P, NT], f32, tag="g")
            g_bf = st_pool.tile([P, NT], bf16, tag="gbf")
            e_tiles = []

            def make_tail(h=h, c_ps=c_ps, v_s=v_s, last_head=last_head):
                def tail():
                    # colsum [4 x 512 at partitions 0/32/64/96] -> csT [P, NT].
                    # m = 16p + t, so chunk-major cs4 [4x512] and csT [128,16]
                    # walk m in the same order: one direct SBUF->SBUF DMA.
                    cs4 = st_pool.tile([P, 512], f32, tag="cs4")
                    nc.vector.tensor_copy(cs4[:, :], c_ps[:, :])
                    csT = st_pool.tile([P, NT], f32, tag="csT")
                    nc.sync.dma_start(out=csT[:, :], in_=cs4[0:P:32, :])
                    o_s = vo_pool.tile([P, NT, D], f32, tag="o")
                    out_r = out[h].rearrange("(p t) d -> p t d", p=P)
                    if last_head:
                        # split the exposed tail across DVE + GpSimd so the
                        # multiply and the two stores overlap; the GpSimd
                        # share is smaller since it runs ~2.4x slower
                        parts = ((nc.vector, 0, 8), (nc.gpsimd, 8, NT))
                    else:
                        parts = ((nc.gpsimd, 0, NT),)
                    for eng, t0, t1 in parts:
                        eng.tensor_tensor(
                            o_s[:, t0:t1, :],
                            v_s[:, t0:t1, :],
                            csT[:, t0:t1].unsqueeze(-1).broadcast_to((P, t1 - t0, D)),
                            op=Mult,
                        )
                        nc.sync.dma_start(
                            out=out_r[:, t0:t1, :], in_=o_s[:, t0:t1, :]
                        )

                return tail

            tail_fn = make_tail()

            # g-batches: pairs, so g deliveries are fine-grained and the
            # colsum stream stays continuous; the last head finishes with
            # micro-batches so colsum work drains in-loop.
            dve_tiles = DVE_TILES_LAST if last_head else DVE_TILES
            mixed_tiles = frozenset() if last_head else MIXED_TILES
            batches = [(b0, b0 + 1) for b0 in range(0, NT - 2, 2)]
            if last_head:
                batches += [(14, 14), (15, 15)]
            else:
                batches += [(14, 15)]
            batch_of = {}
            for b0, b1 in batches:
                for j in range(b0, b1 + 1):
                    batch_of[j] = (b0, b1)

            # rowsum state per batch: slot at which the batch's parts are all
            # emitted (used to derive the colsum drip lag)
            dve_in_batch = {
                (b0, b1): [j for j in range(b0, b1 + 1) if j in dve_tiles]
                for b0, b1 in batches
            }

            # --- per-tile emission machinery -------------------------
            # DVE tiles get a dedicated single S buffer (sd_pool) so the
            # ACT exp relay never threads through a DVE exp via PSUM WAR;
            # the DVE tile's second m-half is deferred one slot so its fill
            # (which WARs the first half's exp) never parks the PE queue.
            halves_emitted = {}
            reds_emitted = set()

            def emit_half(i, mh, e_i, tmp, use_dve):
                half_dve = use_dve or (i in mixed_tiles and mh == 1)
                pool = sd_pool if half_dve else sa_pool
                s_ps = pool.tile([P, MW], f32, tag="sd" if half_dve else "sa",
                                 name="s_ps")
                for c in range(MW // 512):
                    m0 = mh * MW + c * 512
                    nc.tensor.matmul(
                        s_ps[:, c * 512 : (c + 1) * 512],
                        lhsT=q_s[:, :, i * P : (i + 1) * P],
                        rhs=k_s[:, :, m0 : m0 + 512],
                        start=True,
                        stop=True,
                        perf_mode=DoubleRow,
                    )
                if half_dve:
                    nc.vector.tensor_scalar(
                        e_i[:, mh * MW : (mh + 1) * MW].bitcast(i16),
                        s_ps[:, :],
                        SCH_A,
                        SCH_B,
                        op0=Mult,
                        op1=Add,
                    )
                    # GpSimd folds each half as soon as it lands, so the
                    # row-sum reduce is over 1024 elements, not 2048
                    h0 = mh * MW
                    nc.gpsimd.tensor_tensor(
                        tmp[:, mh, :],
                        e_i[:, h0 : h0 + 512],
                        e_i[:, h0 + 512 : h0 + MW],
                        op=Add,
                    )
                    if not use_dve:
                        # mixed tile: reduce the folded second half into the
                        # rs_parts slot the accumulator didn't fill
                        def emit_mreduce(i=i, tmp=tmp, rs_parts=rs_parts,
                                         reds_emitted=reds_emitted):
                            nc.vector.tensor_reduce(
                                rs_parts[:, i, 1:2],
                                tmp[:, 1, :],
                                axis=mybir.AxisListType.X,
                                op=Add,
                            )
                            reds_emitted.add(i)

                        red_pending.append((slot + 2, emit_mreduce))
                else:
                    nc.scalar.activation(
                        e_i[:, mh * MW : (mh + 1) * MW],
                        s_ps[:, :],
                        func=Exp,
                        scale=SCALE,
                        accum_out=rs_parts[:, i, mh : mh + 1],
                    )
                halves_emitted[i] = halves_emitted.get(i, 0) + 1
                if use_dve and mh == 1:
                    def emit_reduce(i=i, tmp=tmp, rowsum=rowsum,
                                    reds_emitted=reds_emitted):
                        nc.vector.tensor_reduce(
                            rowsum[:, i : i + 1],
                            tmp[:, :, :],
                            axis=mybir.AxisListType.XY,
                            op=Add,
                        )
                        reds_emitted.add(i)

                    red_pending.append((slot + 2, emit_reduce))

            # bind per-head state as defaults: the head loop shares one
            # Python scope, and a boundary can flush during the next head
            def emit_boundary(b0, b1, rowsum=rowsum, rs_parts=rs_parts, g=g,
                              g_bf=g_bf, dve_tiles=dve_tiles, c_ps=c_ps,
                              e_tiles=e_tiles, tail_fn=tail_fn,
                              last_head=last_head):
                sl = slice(b0, b1 + 1)
                act_js = [j for j in range(b0, b1 + 1) if j not in dve_tiles]
                if act_js and act_js == list(range(act_js[0], b1 + 1)):
                    asl = slice(act_js[0], b1 + 1)
                    nc.vector.tensor_tensor(
                        rowsum[:, asl],
                        rs_parts[:, asl, 0],
                        rs_parts[:, asl, 1],
                        op=Add,
                    )
                else:
                    for j in act_js:
                        nc.vector.tensor_tensor(
                            rowsum[:, j : j + 1],
                            rs_parts[:, j : j + 1, 0],
                            rs_parts[:, j : j + 1, 1],
                            op=Add,
                        )
                nc.vector.reciprocal(g[:, sl], rowsum[:, sl])
                nc.vector.tensor_copy(g_bf[:, sl], g[:, sl])
                # colsum drip lag: a parked colsum head-of-line-blocks the S
                # fills queued behind it, so defer far past the g chain; the
                # last head drains sooner (parking is harmless once its own
                # fills are done).
                lag = 3 if last_head else 5
                for idx, j in enumerate(range(b0, b1 + 1)):
                    pending.append(
                        [
                            slot + lag + idx,
                            0,
                            j,
                            c_ps,
                            g_bf,
                            e_tiles[j],
                            tail_fn if j == NT - 1 else None,
                        ]
                    )

            def make_ready(b0, b1, halves_emitted=halves_emitted,
                           reds_emitted=reds_emitted, dve_tiles=dve_tiles,
                           mixed_tiles=mixed_tiles):
                def ready():
                    if any(halves_emitted.get(j, 0) < 2
                           for j in range(b0, b1 + 1)):
                        return False
                    gated = dve_tiles | mixed_tiles
                    return not any(j in gated and j not in reds_emitted
                                   for j in range(b0, b1 + 1))

                return ready

            for _b0, _b1 in batches:
                bnd_pending.append(
                    (make_ready(_b0, _b1),
                     lambda b0=_b0, b1=_b1, f=emit_boundary: f(b0, b1))
                )

            deferred_mh1 = []

            for i in range(NT):
                slot += 1
                use_dve = i in dve_tiles
                e_i = e_pool.tile([P, N], bf16, tag="e")
                e_tiles.append(e_i)
                tmp = None
                if use_dve or i in mixed_tiles:
                    tmp = tmp_pool.tile([P, 2, 512], bf16, tag="t", name="tmp")

                emit_half(i, 0, e_i, tmp, use_dve)
                drip_colsum(DRIP0)
                # the previous DVE tile's deferred second half
                while deferred_mh1:
                    deferred_mh1.pop(0)()
                if (use_dve or i in mixed_tiles) and i < NT - 1:
                    deferred_mh1.append(
                        lambda i=i, e_i=e_i, tmp=tmp, d=use_dve: emit_half(
                            i, 1, e_i, tmp, d
                        )
                    )
                else:
                    emit_half(i, 1, e_i, tmp, use_dve)
                drip_colsum(6 if len(pending) > 2 else DRIP1)

                # flush due row-sum reduces, then any completed g-pairs
                while red_pending and red_pending[0][0] <= slot:
                    red_pending.pop(0)[1]()
                flush_boundaries()

            # end of head: drain this head's remaining bookkeeping next head
            # (closures capture this head's tiles); the last head forces it
            if last_head:
                while deferred_mh1:
                    deferred_mh1.pop(0)()
                while red_pending:
                    red_pending.pop(0)[1]()
                flush_boundaries()
                assert not bnd_pending, "unflushed g-pair boundaries"

            if last_head:
                while red_pending:
                    red_pending.pop(0)[1]()
                slot += NT  # make everything due
                drip_colsum(10**9)


_NC_CACHE = None


def _get_nc():
    global _NC_CACHE
    if _NC_CACHE is None:
        nc = bacc.Bacc("TRN2", target_bir_lowering=False, debug=False)
        qT = nc.dram_tensor(
            "qT", [H_LOC, D // 2, 2, N], f8e4, kind="ExternalInput"
        ).ap()
        kT = nc.dram_tensor(
            "kT", [H_LOC, D // 2, 2, N], f8e4, kind="ExternalInput"
        ).ap()
        vin = nc.dram_tensor("v", [H_LOC, N, D], f32, kind="ExternalInput").ap()
        out = nc.dram_tensor("out", [H_LOC, N, D], f32, kind="ExternalOutput").ap()
        with tile.TileContext(nc) as tc:
            _attention_kernel(tc, out, qT, kT, vin)
        nc.compile()
        _NC_CACHE = nc
    return _NC_CACHE


def kernel(q, k, v):
    q = np.asarray(q, dtype=np.float32).reshape(B * H, N, D)
    k = np.asarray(k, dtype=np.float32).reshape(B * H, N, D)
    v = np.asarray(v, dtype=np.float32).reshape(B * H, N, D)

    f8np = mybir.dt.np(f8e4)
    in_maps = []
    for c in range(N_CORES):
        sl = slice(H_LOC * c, H_LOC * (c + 1))
        qT8 = (
            np.ascontiguousarray(q[sl].transpose(0, 2, 1))
            .astype(f8np)
            .reshape(H_LOC, D // 2, 2, N)
        )
        kT8 = (
            np.ascontiguousarray(k[sl].transpose(0, 2, 1))
            .astype(f8np)
            .reshape(H_LOC, D // 2, 2, N)
        )
        in_maps.append({"qT": qT8, "kT": kT8, "v": np.ascontiguousarray(v[sl])})

    trace = bool(os.environ.get("KERNEL_TRACE"))
    res = run_bass_kernel_spmd(
        _get_nc(), in_maps, core_ids=list(range(N_CORES)), trace=trace
    )
    if trace:
        print(f"HW exec time: {res.exec_time_ns} ns")
        if res.instructions_and_trace is not None:
            print(f"trace: {res.instructions_and_trace[1]}")

    outs = [r["out"] for r in res.results]
    return np.concatenate(outs, axis=0).reshape(B, H, N, D)
